# revision 26
# baseline (speedup 1.0000x reference)
"""Causal self-attention (dense transformer block) on 8 Trainium2 NeuronCores.

Tensor-parallel over heads: 16 heads / 8 cores = 2 heads per core.
Each core:
  qkvT  = Wqkv_shard @ x.T  (+bias)          [384, 4096]   (q/k/v for its 2 heads)
  RoPE on qT,kT (rotation via PE perm-matmul + DVE/GPSIMD elementwise)
  flash-style causal attention in "transposed" layout:
      scoresT[k, q] = k . q    (PE, contraction over head dim)
      pT = exp(scoresT/8)      (ACT, straight from PSUM)
      [sums; outT] += [ones|v].T @ pT   (PE, fused denominator row)
      outT /= sums             (PE broadcast-matmul + DVE)
  AllGather of outT shards (rank-major rows == channel order)
  y_slice = proj_w_shard @ outT_all (+bias)  -> each core owns 128 output cols

Host side: shard/transpose weights per core, run SPMD, concat slices.
"""

import math

import numpy as np

B = 2
T = 2048
C = 1024
H = 16
DH = 64
N_CORES = 8
HPC = H // N_CORES  # heads per core = 2
QC_W = 1024         # q-chunk width for attention
NKT = T // 128      # 16 k-tiles per sequence

# 'float32r' (full-rate fp32 matmul mode) or 'float32' (4x slower, exact)
MM_DTYPE = "float32r"

_BUILD_CACHE = {}


def _mk_host_consts():
    inv_freq = 1.0 / (10000.0 ** (np.arange(0, DH, 2, dtype=np.float64) / DH))
    t = np.arange(T, dtype=np.float64)
    freqs = np.outer(t, inv_freq)                    # [T, 32]
    emb = np.concatenate([freqs, freqs], axis=-1)    # [T, 64]
    emb32 = emb.astype(np.float32).astype(np.float64)
    cosT = np.cos(emb32).T.astype(np.float32)        # [64, T]
    sinT = np.sin(emb32).T.astype(np.float32)
    cosT2 = np.concatenate([cosT, cosT], axis=0)     # [128, T] two heads stacked
    sinT2 = np.concatenate([sinT, sinT], axis=0)

    # P2 @ q2h == rotate_half applied per head (rows 0-63 head0, 64-127 head1)
    R = np.zeros((DH, DH), dtype=np.float32)
    half = DH // 2
    for d in range(half):
        R[d, d + half] = -1.0
    for d in range(half, DH):
        R[d, d - half] = 1.0
    P2 = np.zeros((128, 128), dtype=np.float32)
    P2[:DH, :DH] = R
    P2[DH:, DH:] = R
    protT = np.ascontiguousarray(P2.T)

    ident2 = np.concatenate([np.eye(64, dtype=np.float32)] * 2, axis=0)  # [128, 64]
    # mask01[kk, qq] = 1 if qq >= kk else 0  (columns = q offset within diag block)
    mask01 = np.triu(np.ones((128, 128), dtype=np.float32))
    ones1 = np.ones((128, 64), dtype=np.float32)
    return cosT2, sinT2, protT, ident2, mask01, ones1


def _build(mm_dtype=MM_DTYPE, variant="full", repeat=1):
    """Build the Bass module (shared SPMD program for all 8 cores).

    variant: 'full' | 'noproj' (skip AG+proj) | 'noattn' (qkv/rope/v only)
             | 'dvemask' (diag masks + rope cos-mul on DVE instead of GPSIMD)
    repeat: emit the whole computation N times (device-time measurement).
    """
    from contextlib import ExitStack

    import concourse.bacc as bacc
    import concourse.mybir as mybir
    from concourse.tile import TileContext

    f32 = mybir.dt.float32
    mdt = getattr(mybir.dt, mm_dtype)
    AF = mybir.ActivationFunctionType
    ALU = mybir.AluOpType

    nc = bacc.Bacc("TRN2", target_bir_lowering=False, debug=False,
                   num_devices=N_CORES)

    # ---- kernel I/O ----
    xt_d = nc.dram_tensor("xt", [B, C, T], mdt, kind="ExternalInput").ap()
    wqkvT_d = nc.dram_tensor("wqkvT", [C, 3 * 128], mdt, kind="ExternalInput").ap()
    bqkvT_d = nc.dram_tensor("bqkvT", [3 * 128, 1], f32, kind="ExternalInput").ap()
    cosT_d = nc.dram_tensor("cosT2", [128, T], f32, kind="ExternalInput").ap()
    sinT_d = nc.dram_tensor("sinT2", [128, T], f32, kind="ExternalInput").ap()
    protT_d = nc.dram_tensor("protT", [128, 128], mdt, kind="ExternalInput").ap()
    ident_d = nc.dram_tensor("ident2", [128, 64], f32, kind="ExternalInput").ap()
    mask_d = nc.dram_tensor("mask01", [128, 128], mdt, kind="ExternalInput").ap()
    ones_d = nc.dram_tensor("ones1", [128, 64], mdt, kind="ExternalInput").ap()
    pwT_d = nc.dram_tensor("pwT", [C, 128], mdt, kind="ExternalInput").ap()
    pbias_d = nc.dram_tensor("pbias", [128, 1], f32, kind="ExternalInput").ap()
    y_d = nc.dram_tensor("y", [B, T, 128], f32, kind="ExternalOutput").ap()

    def m(ap):
        return ap

    with TileContext(nc) as tc, ExitStack() as ctx:
        cpool = ctx.enter_context(tc.tile_pool(name="consts", bufs=1))
        xpool = ctx.enter_context(tc.tile_pool(name="xt", bufs=9))
        qkvpool = ctx.enter_context(tc.tile_pool(name="qkv", bufs=1))
        rpool = ctx.enter_context(tc.tile_pool(name="rope", bufs=2))
        tpool = ctx.enter_context(tc.tile_pool(name="ropetmp", bufs=2))
        vpool = ctx.enter_context(tc.tile_pool(name="vn", bufs=4))
        ptpool = ctx.enter_context(tc.tile_pool(name="pt", bufs=3))
        opool = ctx.enter_context(tc.tile_pool(name="outT", bufs=3))
        epool = ctx.enter_context(tc.tile_pool(name="epi", bufs=2))
        agpool = ctx.enter_context(tc.tile_pool(name="ag", bufs=8))
        ypool = ctx.enter_context(tc.tile_pool(name="y", bufs=3))
        ps_mm = ctx.enter_context(tc.tile_pool(name="psmm", bufs=2, space="PSUM"))
        ps_sc = ctx.enter_context(tc.tile_pool(name="pssc", bufs=2, space="PSUM"))
        ps_acc = ctx.enter_context(tc.tile_pool(name="psacc", bufs=1, space="PSUM"))
        dpool = ctx.enter_context(tc.tile_pool(name="dram", bufs=1, space="DRAM"))

        # ---- load constants ----
        w_sb = []
        for ct in range(8):
            w = cpool.tile([128, 384], mdt, name=f"w{ct}")
            nc.sync.dma_start(w[:], wqkvT_d[ct * 128:(ct + 1) * 128, :])
            w_sb.append(w)
        pw_sb = []
        for ct in range(8):
            pw = cpool.tile([128, 128], mdt, name=f"pw{ct}")
            nc.sync.dma_start(pw[:], pwT_d[ct * 128:(ct + 1) * 128, :])
            pw_sb.append(pw)
        bq_sb = []
        for mt in range(3):
            bqt = cpool.tile([128, 1], f32, name=f"bq{mt}")
            nc.sync.dma_start(bqt[:], bqkvT_d[mt * 128:(mt + 1) * 128, :])
            bq_sb.append(bqt)
        cos_sb = cpool.tile([128, T], f32, name="cos_sb")
        nc.sync.dma_start(cos_sb[:], cosT_d[:])
        sin_sb = cpool.tile([128, T], f32, name="sin_sb")
        nc.sync.dma_start(sin_sb[:], sinT_d[:])
        prot_sb = cpool.tile([128, 128], mdt, name="prot_sb")
        nc.sync.dma_start(prot_sb[:], protT_d[:])
        id_sb = cpool.tile([128, 64], f32, name="id_sb")
        nc.sync.dma_start(id_sb[:], ident_d[:])
        mask_sb = cpool.tile([128, 128], mdt, name="mask_sb")
        nc.sync.dma_start(mask_sb[:], mask_d[:])
        ones_sb = cpool.tile([128, 64], mdt, name="ones_sb")
        nc.sync.dma_start(ones_sb[:], ones_d[:])
        pb_sb = cpool.tile([128, 1], f32, name="pb_sb")
        nc.sync.dma_start(pb_sb[:], pbias_d[:])

        qp = {}   # rope'd qT per batch  [128, T]
        kp = {}
        vn = {}   # (b, h) -> [128, NKT*65] ones|v tiles
        outT = {}
        cc_out = {}

        def phase_qkv_rope_v(b):
            q2h = qkvpool.tile([128, T], mdt, name=f"q2h_b{b}", tag="q2h")
            k2h = qkvpool.tile([128, T], mdt, name=f"k2h_b{b}", tag="k2h")
            v2h = qkvpool.tile([128, T], f32, name=f"v2h_b{b}", tag="v2h")
            dests = [q2h, k2h, v2h]
            for tcn in range(4):
                ts = tcn * 512
                xts = []
                for ct in range(8):
                    xtile = xpool.tile([128, 512], mdt, name=f"xt_b{b}_{tcn}_{ct}",
                                       tag="xt")
                    nc.sync.dma_start(xtile[:], xt_d[b, ct * 128:(ct + 1) * 128,
                                                    ts:ts + 512])
                    xts.append(xtile)
                for mt in range(3):
                    ps = ps_mm.tile([128, 512], f32, name=f"psqkv_{b}_{tcn}_{mt}",
                                    tag="mm")
                    for ct in range(8):
                        nc.tensor.matmul(
                            ps[:], m(w_sb[ct][:, mt * 128:(mt + 1) * 128]),
                            m(xts[ct][:]), start=(ct == 0), stop=(ct == 7))
                    # PSUM -> SBUF copy with per-partition bias add
                    nc.scalar.activation(dests[mt][:, ts:ts + 512], ps[:],
                                         AF.Identity, bias=bq_sb[mt][:])

            # --- RoPE ---
            for nm, src, dstmap in (("q", q2h, qp), ("k", k2h, kp)):
                dst = rpool.tile([128, T], mdt, name=f"{nm}p_b{b}", tag=f"{nm}p")
                for tcn in range(4):
                    ts = tcn * 512
                    rp = ps_mm.tile([128, 512], f32, name=f"psrot_{nm}_{b}_{tcn}",
                                    tag="mm")
                    nc.tensor.matmul(rp[:], m(prot_sb[:]), m(src[:, ts:ts + 512]),
                                     start=True, stop=True)
                    t1 = tpool.tile([128, 512], f32, name=f"t1_{nm}_{b}_{tcn}",
                                    tag="t1")
                    nc.vector.tensor_mul(t1[:], rp[:], sin_sb[:, ts:ts + 512])
                    t2 = tpool.tile([128, 512], f32, name=f"t2_{nm}_{b}_{tcn}",
                                    tag="t2")
                    nc.vector.tensor_mul(t2[:], src[:, ts:ts + 512],
                                         cos_sb[:, ts:ts + 512])
                    nc.vector.tensor_add(dst[:, ts:ts + 512], t2[:], t1[:])
                dstmap[b] = dst

            # --- v transpose into [v | ones] tiles (v cols 0-63, ones col 64) ---
            for h in range(HPC):
                vt = vpool.tile([128, NKT * 65], mdt, name=f"vn_b{b}_h{h}", tag="vn")
                ones_view = vt[:].rearrange("p (k c) -> p k c", c=65)[:, :, 64:65]
                nc.gpsimd.memset(ones_view.bitcast(f32), 1.0)
                for g in range(4):  # groups of 4 k-tiles
                    tp = ps_mm.tile([128, 256], f32, name=f"psvt_{b}_{h}_{g}",
                                    tag="mm")
                    for j in range(4):
                        kt = g * 4 + j
                        nc.tensor.transpose(
                            tp[:, j * 64:(j + 1) * 64],
                            v2h[h * 64:(h + 1) * 64, kt * 128:(kt + 1) * 128],
                            id_sb[h * 64:(h + 1) * 64, :])
                    dstv = (vt[:, g * 4 * 65:(g + 1) * 4 * 65]
                            .rearrange("p (k c) -> p k c", c=65)[:, :, 0:64])
                    srcv = tp[:].rearrange("p (k c) -> p k c", c=64)
                    nc.vector.tensor_copy(dstv, srcv)
                vn[(b, h)] = vt

        def phase_attn(b, qc):
            """Attention for both heads of batch b, q range [qc*QC_W, (qc+1)*QC_W)."""
            qs = qc * QC_W
            for h in range(HPC):
                ot = opool.tile([64, QC_W], mdt, name=f"outT_b{b}_h{h}_q{qc}",
                                tag="outT")
                outT[(b, h, qc)] = ot
                hr = slice(h * 64, (h + 1) * 64)
                nkt = (qs + QC_W) // 128
                acc = ps_acc.tile([65, QC_W], f32, name=f"acc_{b}_{h}_{qc}",
                                  tag="acc")
                pending = None

                def emit_av(p, acc=acc, b=b, h=h, nkt=nkt):
                    kt, off, pt = p
                    for a, bb in _halves(off, QC_W):
                        nc.tensor.matmul(
                            acc[:, a:bb], m(vn[(b, h)][:, kt * 65:(kt + 1) * 65]),
                            m(pt[:, a - off:bb - off]),
                            start=(kt == 0), stop=(kt == nkt - 1),
                            skip_group_check=True)

                for kt in range(nkt):
                    off = max(0, kt * 128 - qs)
                    vc = QC_W - off
                    sc = ps_sc.tile([128, QC_W], f32, name=f"sc_{b}_{h}_{qc}_{kt}",
                                    tag="sc")
                    for a, bb in _halves(off, QC_W):
                        nc.tensor.matmul(
                            sc[:, a:bb],
                            m(kp[b][hr, kt * 128:(kt + 1) * 128]),
                            m(qp[b][hr, qs + a:qs + bb]),
                            start=True, stop=True, skip_group_check=True)
                    pt = ptpool.tile([128, QC_W], mdt, name=f"pt_{b}_{h}_{qc}_{kt}",
                                     tag="pt")
                    nc.scalar.activation(pt[:, 0:vc], sc[:, off:QC_W], AF.Exp,
                                         scale=1.0 / math.sqrt(DH))
                    if kt * 128 >= qs:
                        # diagonal tile: zero strictly-below-diagonal entries
                        nc.vector.tensor_mul(pt[:, 0:128], pt[:, 0:128], mask_sb[:])
                    if pending is not None:
                        emit_av(pending)
                    pending = (kt, off, pt)
                emit_av(pending)

                # epilogue: normalize.  acc row 64 = sums; rows 0-63 = outT.
                rec = epool.tile([65, QC_W], mdt, name=f"rec_{b}_{h}_{qc}",
                                 tag="rec")
                with nc.allow_low_precision(reason="softmax recip feeds fp32r mm"):
                    nc.vector.reciprocal(rec[64:65, :], acc[64:65, :])
                rb = ps_sc.tile([64, QC_W], f32, name=f"rb_{b}_{h}_{qc}", tag="sc")
                for a in range(0, QC_W, 512):
                    nc.tensor.matmul(rb[:, a:a + 512], m(ones_sb[64:65, :]),
                                     m(rec[64:65, a:a + 512]), start=True,
                                     stop=True, skip_group_check=True)
                rbs = epool.tile([64, QC_W], f32, name=f"rbs_{b}_{h}_{qc}",
                                 tag="rbs", bufs=1)
                nc.scalar.copy(rbs[:], rb[:])
                nc.vector.tensor_mul(ot[:], acc[0:64, :], rbs[:])

        def emit_allgather(b, qc):
            qs = qc * QC_W
            cc_in = dpool.tile([128, QC_W], mdt, name=f"ccin_{b}_{qc}")
            nc.sync.dma_start(cc_in[0:64, :], outT[(b, 0, qc)][:])
            nc.sync.dma_start(cc_in[64:128, :], outT[(b, 1, qc)][:])
            cco = dpool.tile([C, QC_W], mdt, name=f"ccout_{b}_{qc}",
                             addr_space="Shared" if variant != "projnoag" else "Local")
            if variant != "projnoag":
                nc.gpsimd.collective_compute(
                    "AllGather", mybir.AluOpType.bypass,
                    replica_groups=[list(range(N_CORES))],
                    ins=[cc_in[:]], outs=[cco[:]])
            cc_out[(b, qc)] = cco

        def phase_proj(b, qc):
            cco = cc_out[(b, qc)]
            for half in range(QC_W // 512):
                ts = half * 512          # token offset within this qc chunk
                gts = qc * QC_W + ts     # token offset within batch
                ags = []
                for ct in range(8):
                    ag = agpool.tile([128, 512], mdt,
                                     name=f"ag_{b}_{qc}_{half}_{ct}", tag="ag")
                    nc.sync.dma_start(ag[:], cco[ct * 128:(ct + 1) * 128,
                                              ts:ts + 512])
                    ags.append(ag)
                psy = ps_mm.tile([128, 512], f32, name=f"psy_{b}_{qc}_{half}",
                                 tag="mm")
                for ct in range(8):
                    nc.tensor.matmul(psy[:], m(pw_sb[ct][:]), m(ags[ct][:]),
                                     start=(ct == 0), stop=(ct == 7))
                ysb = ypool.tile([128, 512], f32, name=f"y_{b}_{qc}_{half}",
                                 tag="y")
                nc.vector.tensor_scalar_add(ysb[:], psy[:], pb_sb[:])
                nc.sync.dma_start(
                    y_d[b, gts:gts + 512, :].rearrange("t o -> o t"), ysb[:])

        # ---------------- emission order ----------------
        for _rep in range(repeat):
            phase_qkv_rope_v(0)
            phase_qkv_rope_v(1)
            if variant != "noattn":
                phase_attn(0, 0)
                if variant != "noproj":
                    emit_allgather(0, 0)
                phase_attn(0, 1)
                if variant != "noproj":
                    emit_allgather(0, 1)
                    phase_proj(0, 0)
                phase_attn(1, 0)
                if variant != "noproj":
                    emit_allgather(1, 0)
                    phase_proj(0, 1)
                phase_attn(1, 1)
                if variant != "noproj":
                    emit_allgather(1, 1)
                    phase_proj(1, 0)
                    phase_proj(1, 1)  # noqa

    nc.compile()
    return nc


def _halves(off, w):
    """Split column range [off, w) at 512-boundaries into (a, b) pieces."""
    out = []
    a = off
    while a < w:
        b = min((a // 512 + 1) * 512, w)
        out.append((a, b))
        a = b
    return out


def _make_in_maps(x, qkv_w, qkv_b, proj_w, proj_b):
    cosT2, sinT2, protT, ident2, mask01, ones1 = _mk_host_consts()
    xt = np.ascontiguousarray(x.transpose(0, 2, 1)).astype(np.float32)
    in_maps = []
    for d in range(N_CORES):
        h0, h1 = 2 * d, 2 * d + 1
        rows = []
        for blk in range(3):  # q, k, v
            for h in (h0, h1):
                rows.append(np.arange(blk * C + h * DH, blk * C + (h + 1) * DH))
        rows = np.concatenate(rows)
        wsel = qkv_w[rows, :]                    # [384, 1024]
        bsel = qkv_b[rows].reshape(384, 1)
        in_maps.append({
            "xt": xt,
            "wqkvT": np.ascontiguousarray(wsel.T).astype(np.float32),
            "bqkvT": np.ascontiguousarray(bsel).astype(np.float32),
            "cosT2": cosT2, "sinT2": sinT2, "protT": protT,
            "ident2": ident2, "mask01": mask01, "ones1": ones1,
            "pwT": np.ascontiguousarray(proj_w[d * 128:(d + 1) * 128, :].T)
                     .astype(np.float32),
            "pbias": np.ascontiguousarray(proj_b[d * 128:(d + 1) * 128]
                                          .reshape(128, 1)).astype(np.float32),
        })
    return in_maps


def kernel(x, qkv_w, qkv_b, proj_w, proj_b):
    x = np.asarray(x, dtype=np.float32)
    qkv_w = np.asarray(qkv_w, dtype=np.float32)
    qkv_b = np.asarray(qkv_b, dtype=np.float32)
    proj_w = np.asarray(proj_w, dtype=np.float32)
    proj_b = np.asarray(proj_b, dtype=np.float32)

    if "nc" not in _BUILD_CACHE:
        _BUILD_CACHE["nc"] = _build()
    nc = _BUILD_CACHE["nc"]
    in_maps = _make_in_maps(x, qkv_w, qkv_b, proj_w, proj_b)

    from concourse.bass_utils import run_bass_kernel_spmd
    res = run_bass_kernel_spmd(nc, in_maps, core_ids=list(range(N_CORES)))
    y = np.concatenate([res.results[d]["y"] for d in range(N_CORES)], axis=-1)
    return y.astype(np.float32)
